# revision 19
# baseline (speedup 1.0000x reference)
"""Trainium2 Bass kernel for CrossAttentionInjection (block-diagonal frame attention).

Contract: kernel(**inputs) takes FULL unsharded numpy inputs (as produced by
setup_inputs()) and returns the FULL [B, T, Q_DIM] float32 output.

Sharding: the attention mask is block-diagonal over 8 frames x 256 patches, so
the whole module decomposes into 32 independent (batch, frame) blocks of 256
tokens. Each of the 8 cores processes 4 contiguous blocks (1024 tokens of one
batch) with replicated weights -- no collectives.
"""

import numpy as np

# ---------------------------------------------------------------------------
# Problem constants (hardcoded; kernel.py must be self-contained)
# ---------------------------------------------------------------------------
B, T, Q_DIM, KV_DIM = 4, 2048, 1024, 768
HEADS, DIM_HEAD = 16, 64
INNER = HEADS * DIM_HEAD  # 1024
NUM_FRAMES, NUM_PATCHES = 8, 256
LN_EPS = 1e-5
N_CORES = 8
TOK = B * T // N_CORES          # 1024 tokens per core
NB = TOK // NUM_PATCHES         # 4 frame-blocks per core
BLK = NUM_PATCHES               # 256
DT = Q_DIM // 128               # 8 q-dim partition tiles
DKT = KV_DIM // 128             # 6 kv-dim partition tiles
SCALE = DIM_HEAD ** -0.5        # 0.125

_CACHE = {}


def _patch_tile_drain():
    """This walrus build rejects >1 sync-wait on a Drain CTRL instruction.
    Split the Tile end-of-context drain waits across single-wait NOPs."""
    import concourse.tile as tile
    from concourse import mybir
    from concourse.vector_clock import ScopedClock

    if getattr(tile.TileContext, "_drain_patched", False):
        return

    def _drain_and_barrier(self, tick_clock, wait_clock):
        nc = self.nc
        probe = nc.sync.nop(nofuse=True)
        wait_clock.add_sem_waits(
            probe.ins, ScopedClock({None: tick_clock.global_clock})
        )
        si = probe.ins.sync_info
        waits = list(si.on_wait) if si is not None else []
        if waits:
            probe.ins.sync_info = mybir.SyncInfo(on_wait=[waits[0]], on_update=[])
            for w in waits[1:]:
                n = nc.sync.nop(nofuse=True)
                n.ins.sync_info = mybir.SyncInfo(on_wait=[w], on_update=[])
        nc.sync.drain()
        nc.all_engine_barrier()
        assert self.sems is not None
        popped = nc._tile_sem_poison_stack.pop()
        assert popped is self._sem_poison
        nc.clear_and_free_semaphores(list(self.sems.allocated().values()))
        nc.all_engine_barrier()

    tile.TileContext._drain_and_barrier = _drain_and_barrier
    tile.TileContext._drain_patched = True


def _split_multi_waits(nc, mybir, max_waits=1):
    """This walrus build accepts at most one sync-wait per instruction.
    Move extra waits onto single-wait NOPs inserted just before, on the
    same engine (sound: same-engine program order is preserved)."""
    ctr = [0]
    for fn in nc.m.functions:
        for blk in fn.blocks:
            new = []
            changed = False
            for inst in blk.instructions:
                si = inst.sync_info
                waits = list(si.on_wait) if si is not None else []
                if len(waits) > max_waits:
                    changed = True
                    for w in waits[:-max_waits]:
                        ctr[0] += 1
                        new.append(mybir.InstNoOp(
                            name=f"I-waitsplit-{ctr[0]}",
                            engine=inst.engine,
                            sync_info=mybir.SyncInfo(on_wait=[w], on_update=[]),
                        ))
                    inst.sync_info = mybir.SyncInfo(
                        on_wait=waits[-max_waits:],
                        on_update=list(si.on_update),
                    )
                new.append(inst)
            if changed:
                blk.instructions = new


def _build_nc():
    import concourse.bass as bass
    import concourse.tile as tile
    from concourse import mybir
    from concourse.masks import make_identity

    _patch_tile_drain()

    f32 = mybir.dt.float32
    f32r = mybir.dt.float32r
    bf16 = mybir.dt.bfloat16

    nc = bass.Bass()

    xT = nc.declare_dram_parameter("xT", [Q_DIM, TOK], bf16, isOutput=False)
    ctxT = nc.declare_dram_parameter("ctxT", [KV_DIM, TOK], bf16, isOutput=False)
    wq = nc.declare_dram_parameter("wq", [Q_DIM, INNER], bf16, isOutput=False)
    wk = nc.declare_dram_parameter("wk", [KV_DIM, INNER], bf16, isOutput=False)
    wv = nc.declare_dram_parameter("wv", [KV_DIM, INNER], bf16, isOutput=False)
    wo = nc.declare_dram_parameter("wo", [INNER, Q_DIM], bf16, isOutput=False)
    wsum_neg = nc.declare_dram_parameter("wsum_neg", [1, INNER], f32r, isOutput=False)
    bias_q = nc.declare_dram_parameter("bias_q", [1, INNER], f32r, isOutput=False)
    bo = nc.declare_dram_parameter("bo", [1, Q_DIM], f32, isOutput=False)
    ones_in = nc.declare_dram_parameter("ones_in", [1, 128], f32r, isOutput=False)
    y = nc.declare_dram_parameter("y", [TOK, Q_DIM], f32, isOutput=True)

    with tile.TileContext(nc) as tc:
        import contextlib

        with contextlib.ExitStack() as ctx:
            singles = ctx.enter_context(tc.tile_pool(name="singles", bufs=1))
            qt_pool = ctx.enter_context(tc.tile_pool(name="qt", bufs=2))
            kt_pool = ctx.enter_context(tc.tile_pool(name="kt", bufs=2))
            v_pool = ctx.enter_context(tc.tile_pool(name="v", bufs=2))
            pt_pool = ctx.enter_context(tc.tile_pool(name="pt", bufs=6))
            osb_pool = ctx.enter_context(tc.tile_pool(name="osb", bufs=4))
            rc_pool = ctx.enter_context(tc.tile_pool(name="rc", bufs=4))
            rows_pool = ctx.enter_context(tc.tile_pool(name="rows", bufs=2))
            rbc_pool = ctx.enter_context(tc.tile_pool(name="rbc", bufs=2))
            xsq_pool = ctx.enter_context(tc.tile_pool(name="xsq", bufs=3))
            y_pool = ctx.enter_context(tc.tile_pool(name="y", bufs=2))
            ps_proj = ctx.enter_context(
                tc.tile_pool(name="ps_proj", bufs=2, space="PSUM")
            )
            ps_rows = ctx.enter_context(
                tc.tile_pool(name="ps_rows", bufs=2, space="PSUM")
            )
            ps_st = ctx.enter_context(tc.tile_pool(name="ps_st", bufs=2, space="PSUM"))
            ps_av = ctx.enter_context(tc.tile_pool(name="ps_av", bufs=2, space="PSUM"))

            # ---- resident inputs -------------------------------------------
            xT_sb = singles.tile([128, DT, TOK], bf16)
            nc.sync.dma_start(
                out=xT_sb, in_=xT.rearrange("(a p) t -> p a t", p=128)
            )
            ctxT_sb = singles.tile([128, DKT, TOK], bf16)
            nc.sync.dma_start(
                out=ctxT_sb, in_=ctxT.rearrange("(a p) t -> p a t", p=128)
            )
            wq_sb = singles.tile([128, DT, INNER], bf16)
            nc.sync.dma_start(out=wq_sb, in_=wq.rearrange("(a p) j -> p a j", p=128))
            wk_sb = singles.tile([128, DKT, INNER], bf16)
            nc.sync.dma_start(out=wk_sb, in_=wk.rearrange("(a p) j -> p a j", p=128))
            wv_sb = singles.tile([128, DKT, INNER], bf16)
            nc.sync.dma_start(out=wv_sb, in_=wv.rearrange("(a p) j -> p a j", p=128))
            wo_sb = singles.tile([128, DT, Q_DIM], bf16)
            nc.sync.dma_start(out=wo_sb, in_=wo.rearrange("(a p) j -> p a j", p=128))
            wsum_sb = singles.tile([1, INNER], f32r)
            nc.sync.dma_start(out=wsum_sb, in_=wsum_neg[:, :])
            biasq_sb = singles.tile([1, INNER], f32r)
            nc.sync.dma_start(out=biasq_sb, in_=bias_q[:, :])
            bo_sb = singles.tile([128, Q_DIM], f32)
            nc.sync.dma_start(out=bo_sb, in_=bo[:, :].to_broadcast([128, Q_DIM]))

            # ---- constants -------------------------------------------------
            ones_inv_d = singles.tile([128, 1], bf16)
            nc.vector.memset(ones_inv_d, 1.0 / Q_DIM)
            ones_col = singles.tile([1, 128], f32r)
            nc.sync.dma_start(out=ones_col, in_=ones_in[:, :])
            eps_sb = singles.tile([1, 1], f32)
            nc.vector.memset(eps_sb, LN_EPS)
            ident = singles.tile([128, 128], bf16)
            make_identity(nc, ident)

            # O^T for the whole core, filled per block, consumed by out-proj
            OT_sb = singles.tile([128, DT, TOK], bf16)

            Exp = mybir.ActivationFunctionType.Exp
            Sqrt = mybir.ActivationFunctionType.Sqrt

            for b in range(NB):
                ts, te = b * BLK, (b + 1) * BLK

                # ---- LN statistics (mean / mean-square rows via ones-matmul)
                rows_ps = ps_rows.tile([1, 512], f32)
                for kt in range(DT):
                    nc.tensor.matmul(
                        rows_ps[:, 0:BLK], ones_inv_d, xT_sb[:, kt, ts:te],
                        start=(kt == 0), stop=(kt == DT - 1),
                    )
                for kt in range(DT):
                    xsq = xsq_pool.tile([128, BLK], bf16)
                    nc.vector.tensor_mul(
                        xsq, xT_sb[:, kt, ts:te], xT_sb[:, kt, ts:te]
                    )
                    nc.tensor.matmul(
                        rows_ps[:, BLK:2 * BLK], ones_inv_d, xsq,
                        start=(kt == 0), stop=(kt == DT - 1),
                    )
                # rows_sb: 0:256 mu | 256:512 musq | 512:768 var | 768:1024 rstd
                # | 1024:1280 rinv(=sqrt(var+eps))
                rows_sb = rows_pool.tile([1, 1280], f32r)
                nc.vector.tensor_copy(rows_sb[:, 0:512], rows_ps)
                mu = rows_sb[:, 0:BLK]
                nc.vector.tensor_mul(rows_sb[:, 512:768], mu, mu)
                nc.vector.tensor_sub(
                    rows_sb[:, 512:768], rows_sb[:, BLK:512], rows_sb[:, 512:768]
                )
                var = rows_sb[:, 512:768]
                rstd = rows_sb[:, 768:1024]
                rinv = rows_sb[:, 1024:1280]
                nc.scalar.activation(rinv, var, Sqrt, bias=eps_sb)
                with nc.allow_low_precision(reason="fp32r rounding for PE"):
                    nc.vector.reciprocal(out=rstd, in_=rinv)

                # broadcast rstd down 128 partitions
                rbc_ps = ps_rows.tile([128, BLK], f32, tag="rows_ps")
                nc.tensor.matmul(
                    rbc_ps, ones_col, rstd,
                    start=True, stop=True,
                )
                rbc_sb = rbc_pool.tile([128, BLK], f32)
                nc.vector.tensor_copy(rbc_sb, rbc_ps)

                # ---- Q projection (LN folded in) ---------------------------
                QT_sb = qt_pool.tile([128, DT, BLK], bf16)
                for jt in range(DT):
                    js = jt * 128
                    qps = ps_proj.tile([128, 512], f32, tag="proj")
                    for kt in range(DT):
                        nc.tensor.matmul(
                            qps[:, 0:BLK],
                            wq_sb[:, kt, js:js + 128], xT_sb[:, kt, ts:te],
                            start=(kt == 0), stop=False,
                        )
                    nc.tensor.matmul(
                        qps[:, 0:BLK],
                        wsum_sb[:, js:js + 128], mu,
                        start=False, stop=False,
                    )
                    nc.tensor.matmul(
                        qps[:, 0:BLK],
                        biasq_sb[:, js:js + 128], rinv,
                        start=False, stop=True,
                    )
                    nc.vector.tensor_mul(QT_sb[:, jt, :], qps[:, 0:BLK], rbc_sb)

                # ---- K^T projection ---------------------------------------
                KT_sb = kt_pool.tile([128, DT, BLK], bf16)
                for jt in range(DT):
                    js = jt * 128
                    kps = ps_proj.tile([128, 512], f32, tag="proj")
                    for kt in range(DKT):
                        nc.tensor.matmul(
                            kps[:, 0:BLK],
                            wk_sb[:, kt, js:js + 128], ctxT_sb[:, kt, ts:te],
                            start=(kt == 0), stop=(kt == DKT - 1),
                        )
                    nc.scalar.copy(KT_sb[:, jt, :], kps[:, 0:BLK])

                # ---- V projection (token-major, 65-strided with ones col) --
                V_sb = v_pool.tile([128, 2, HEADS * 65], bf16)
                nc.gpsimd.memset(
                    V_sb.rearrange("p t (h c) -> p t h c", c=65)[:, :, :, 64:65],
                    1.0,
                )
                for t2t in range(2):
                    cs = ts + t2t * 128
                    for jn in range(2):
                        vps = ps_proj.tile([128, 512], f32, tag="proj")
                        for kt in range(DKT):
                            nc.tensor.matmul(
                                vps,
                                ctxT_sb[:, kt, cs:cs + 128],
                                wv_sb[:, kt, jn * 512:(jn + 1) * 512],
                                start=(kt == 0), stop=(kt == DKT - 1),
                            )
                        nc.vector.tensor_copy(
                            V_sb.rearrange("p t (h c) -> p t h c", c=65)[
                                :, t2t, jn * 8:(jn + 1) * 8, 0:64
                            ],
                            vps.rearrange("p (h c) -> p h c", c=64),
                        )

                # ---- attention --------------------------------------------
                osb = [
                    osb_pool.tile([128, INNER], bf16, name=f"osb{t}", tag="osb")
                    for t in range(2)
                ]
                for hg in range(4):
                    pts = []
                    for hh in range(4):
                        h = hg * 4 + hh
                        jt, po = h // 2, (h % 2) * 64
                        stps = ps_st.tile([128, 512], f32)
                        for t2t in range(2):
                            nc.tensor.matmul(
                                stps[:, t2t * BLK:(t2t + 1) * BLK],
                                KT_sb[po:po + 64, jt, t2t * 128:(t2t + 1) * 128],
                                QT_sb[po:po + 64, jt, :],
                                start=True, stop=True,
                            )
                        pt = pt_pool.tile([128, 512], bf16)
                        nc.scalar.activation(pt, stps, Exp, scale=SCALE)
                        pts.append(pt)
                    for t1t in range(2):
                        avp = ps_av.tile([128, 260], f32)
                        for hh in range(4):
                            h = hg * 4 + hh
                            for t2t in range(2):
                                nc.tensor.matmul(
                                    avp[:, hh * 65:(hh + 1) * 65],
                                    pts[hh][:, t2t * BLK + t1t * 128:
                                            t2t * BLK + (t1t + 1) * 128],
                                    V_sb[:, t2t, h * 65:(h + 1) * 65],
                                    start=(t2t == 0), stop=(t2t == 1),
                                )
                        avp_h = avp.rearrange("p (h c) -> p h c", c=65)
                        rc = rc_pool.tile([128, 4], f32)
                        nc.vector.reciprocal(
                            out=rc,
                            in_=avp_h[:, :, 64:65].rearrange("p h c -> p (h c)"),
                        )
                        rc_b = rc.rearrange("p (h o) -> p h o", o=1).to_broadcast(
                            [128, 4, 64]
                        )
                        nc.vector.tensor_mul(
                            osb[t1t].rearrange("p (h c) -> p h c", c=64)[
                                :, hg * 4:(hg + 1) * 4, :
                            ],
                            avp_h[:, :, 0:64],
                            rc_b,
                        )

                # ---- transpose O -> O^T -----------------------------------
                for t1t in range(2):
                    for jt in range(DT):
                        trp = ps_st.tile([128, 128], bf16, tag="stps")
                        nc.tensor.transpose(
                            trp, osb[t1t][:, jt * 128:(jt + 1) * 128], ident
                        )
                        dst = OT_sb[:, jt, ts + t1t * 128: ts + (t1t + 1) * 128]
                        if jt % 2 == 0:
                            nc.vector.tensor_copy(dst, trp)
                        else:
                            nc.scalar.copy(dst, trp)

                # ---- output projection for this block's tokens -------------
                for mtl in range(2):
                    mt = 2 * b + mtl
                    ms = mt * 128
                    y_sb = y_pool.tile([128, Q_DIM], f32)
                    for on in range(2):
                        yps = ps_proj.tile([128, 512], f32, tag="proj")
                        for kt in range(DT):
                            nc.tensor.matmul(
                                yps,
                                OT_sb[:, kt, ms:ms + 128],
                                wo_sb[:, kt, on * 512:(on + 1) * 512],
                                start=(kt == 0), stop=(kt == DT - 1),
                            )
                        nc.vector.tensor_add(
                            y_sb[:, on * 512:(on + 1) * 512],
                            yps,
                            bo_sb[:, on * 512:(on + 1) * 512],
                        )
                    nc.sync.dma_start(out=y[ms:ms + 128, :], in_=y_sb)

    _split_multi_waits(nc, mybir)
    return nc


def _expected_mask():
    fid = np.repeat(np.arange(NUM_FRAMES), NUM_PATCHES)
    return (fid[:, None] == fid[None, :])[None, None]


def _reference_fallback(x, context, ln_gamma, ln_beta, Wq, Wkv, Wo, bo, mask):
    """Pure-numpy fallback for a non-block-diagonal mask (correctness only)."""
    x64 = x.astype(np.float64)
    mu = x64.mean(-1, keepdims=True)
    var = ((x64 - mu) ** 2).mean(-1, keepdims=True)
    xn = (x64 - mu) / np.sqrt(var + LN_EPS) * ln_gamma + ln_beta
    q = xn @ Wq.astype(np.float64)
    kv = context.astype(np.float64) @ Wkv.astype(np.float64)
    k, v = kv[..., :INNER], kv[..., INNER:]
    sh = lambda t: t.reshape(B, T, HEADS, DIM_HEAD).transpose(0, 2, 1, 3)
    q, k, v = sh(q), sh(k), sh(v)
    dots = np.einsum("bhnd,bhmd->bhnm", q, k) * SCALE
    dots = np.where(mask, dots, -np.inf)
    dots -= dots.max(-1, keepdims=True)
    e = np.exp(dots)
    attn = e / e.sum(-1, keepdims=True)
    out = np.einsum("bhnm,bhmd->bhnd", attn, v)
    out = out.transpose(0, 2, 1, 3).reshape(B, T, INNER)
    return (out @ Wo.astype(np.float64) + bo).astype(np.float32)


def _prep_in_maps(x, context, ln_gamma, ln_beta, Wq, Wkv, Wo, bo):
    import ml_dtypes

    bf = ml_dtypes.bfloat16
    wq_eff = (ln_gamma[:, None] * Wq).astype(np.float32)
    wsum_neg = (-wq_eff.sum(axis=0, dtype=np.float64)).astype(np.float32)[None, :]
    bias_q = (ln_beta @ Wq).astype(np.float32)[None, :]
    wk = np.ascontiguousarray(Wkv[:, :INNER]).astype(bf)
    wv = np.ascontiguousarray(Wkv[:, INNER:]).astype(bf)
    wq_b = wq_eff.astype(bf)
    wo_b = Wo.astype(bf)
    bo2 = bo.astype(np.float32)[None, :]

    x_flat = x.reshape(B * T, Q_DIM)
    c_flat = context.reshape(B * T, KV_DIM)
    in_maps = []
    for c in range(N_CORES):
        sl = slice(c * TOK, (c + 1) * TOK)
        in_maps.append({
            "xT": np.ascontiguousarray(x_flat[sl].T).astype(bf),
            "ctxT": np.ascontiguousarray(c_flat[sl].T).astype(bf),
            "wq": wq_b, "wk": wk, "wv": wv, "wo": wo_b,
            "wsum_neg": wsum_neg, "bias_q": bias_q, "bo": bo2,
            "ones_in": np.ones((1, 128), np.float32),
        })
    return in_maps


def _run(inputs, trace=False):
    from concourse.bass_utils import run_bass_kernel_spmd

    if "nc" not in _CACHE:
        _CACHE["nc"] = _build_nc()
    nc = _CACHE["nc"]
    in_maps = _prep_in_maps(
        inputs["x"], inputs["context"], inputs["ln_gamma"], inputs["ln_beta"],
        inputs["Wq"], inputs["Wkv"], inputs["Wo"], inputs["bo"],
    )
    res = run_bass_kernel_spmd(nc, in_maps, list(range(N_CORES)), trace=trace)
    y = np.concatenate([res.results[c]["y"] for c in range(N_CORES)], axis=0)
    return y.reshape(B, T, Q_DIM).astype(np.float32), res


def kernel(x, context, ln_gamma, ln_beta, Wq, Wkv, Wo, bo, mask):
    mask = np.asarray(mask)
    if not np.array_equal(mask, _expected_mask()):
        return _reference_fallback(
            np.asarray(x), np.asarray(context), np.asarray(ln_gamma),
            np.asarray(ln_beta), np.asarray(Wq), np.asarray(Wkv),
            np.asarray(Wo), np.asarray(bo), mask,
        )
    inputs = dict(x=np.asarray(x), context=np.asarray(context),
                  ln_gamma=np.asarray(ln_gamma), ln_beta=np.asarray(ln_beta),
                  Wq=np.asarray(Wq), Wkv=np.asarray(Wkv), Wo=np.asarray(Wo),
                  bo=np.asarray(bo))
    out, _ = _run(inputs, trace=False)
    return out


def _install_profiling_shims():
    """Enable the NTFF profile path under axon in this trimmed container:
    provide the antenv.axon_hooks registry and stub the artifact upload."""
    import sys
    import types

    if "antenv.axon_hooks" not in sys.modules:
        import antenv

        mod = types.ModuleType("antenv.axon_hooks")
        mod._hook = None

        def set_axon_ntff_profile_hook(h):
            mod._hook = h

        def get_axon_ntff_profile_hook():
            return mod._hook

        mod.set_axon_ntff_profile_hook = set_axon_ntff_profile_hook
        mod.get_axon_ntff_profile_hook = get_axon_ntff_profile_hook
        sys.modules["antenv.axon_hooks"] = mod
        antenv.axon_hooks = mod

    mod = sys.modules["antenv.axon_hooks"]
    if mod._hook is None:
        from trn_agent_boot.trn_boot import _ntff_profile_via_ctypes

        mod.set_axon_ntff_profile_hook(
            _ntff_profile_via_ctypes("/opt/axon/libaxon_pjrt.so")
        )

    from concourse import bass_utils

    if not getattr(bass_utils, "_upload_stubbed", False):
        bass_utils.upload_artifacts = lambda tmpdir: tmpdir
        bass_utils._upload_stubbed = True


def kernel_traced(**inputs):
    """Like kernel() but returns (out, BassKernelResults) with profiling."""
    _install_profiling_shims()
    out, res = _run(inputs, trace=True)
    return out, res


# revision 20
# speedup vs baseline: 1.1545x; 1.1545x over previous
"""Trainium2 Bass kernel for CrossAttentionInjection (block-diagonal frame attention).

Contract: kernel(**inputs) takes FULL unsharded numpy inputs (as produced by
setup_inputs()) and returns the FULL [B, T, Q_DIM] float32 output.

Sharding: the attention mask is block-diagonal over 8 frames x 256 patches, so
the whole module decomposes into 32 independent (batch, frame) blocks of 256
tokens. Each of the 8 cores processes 4 contiguous blocks (1024 tokens of one
batch) with replicated weights -- no collectives.

Per-core pipeline (bf16 matmuls, fp32 PSUM, fp32r rank-1 LN corrections):
  LN stats via ones-matmul rows; LN folded into the Q projection (gamma folded
  into Wq on the host, mean/beta as K=1 rank-1 matmuls, 1/std applied in the
  PSUM->SBUF multiply); per-head block attention with exp on ACT (softmax
  scale folded into the activation), denominator via a ones-column appended to
  V; per-partition normalize; PE-transpose of O for the output projection;
  bo added during the final PSUM->SBUF copy.
"""

import numpy as np

# ---------------------------------------------------------------------------
# Problem constants (hardcoded; kernel.py must be self-contained)
# ---------------------------------------------------------------------------
B, T, Q_DIM, KV_DIM = 4, 2048, 1024, 768
HEADS, DIM_HEAD = 16, 64
INNER = HEADS * DIM_HEAD  # 1024
NUM_FRAMES, NUM_PATCHES = 8, 256
LN_EPS = 1e-5
N_CORES = 8
TOK = B * T // N_CORES          # 1024 tokens per core
NB = TOK // NUM_PATCHES         # 4 frame-blocks per core
BLK = NUM_PATCHES               # 256
DT = Q_DIM // 128               # 8 q-dim partition tiles
DKT = KV_DIM // 128             # 6 kv-dim partition tiles
SCALE = DIM_HEAD ** -0.5        # 0.125

_CACHE = {}


def _patch_tile_drain():
    """This walrus build rejects >1 sync-wait on a Drain CTRL instruction.
    Split the Tile end-of-context drain waits across single-wait NOPs."""
    import concourse.tile as tile
    from concourse import mybir
    from concourse.vector_clock import ScopedClock

    if getattr(tile.TileContext, "_drain_patched", False):
        return

    def _drain_and_barrier(self, tick_clock, wait_clock):
        nc = self.nc
        probe = nc.sync.nop(nofuse=True)
        wait_clock.add_sem_waits(
            probe.ins, ScopedClock({None: tick_clock.global_clock})
        )
        si = probe.ins.sync_info
        waits = list(si.on_wait) if si is not None else []
        if waits:
            probe.ins.sync_info = mybir.SyncInfo(on_wait=[waits[0]], on_update=[])
            for w in waits[1:]:
                n = nc.sync.nop(nofuse=True)
                n.ins.sync_info = mybir.SyncInfo(on_wait=[w], on_update=[])
        nc.sync.drain()
        nc.all_engine_barrier()
        assert self.sems is not None
        popped = nc._tile_sem_poison_stack.pop()
        assert popped is self._sem_poison
        nc.clear_and_free_semaphores(list(self.sems.allocated().values()))
        nc.all_engine_barrier()

    tile.TileContext._drain_and_barrier = _drain_and_barrier
    tile.TileContext._drain_patched = True


def _split_multi_waits(nc, mybir, max_waits=1):
    """This walrus build accepts at most one sync-wait per instruction.
    Move extra waits onto single-wait NOPs inserted just before, on the
    same engine (sound: same-engine program order is preserved)."""
    ctr = [0]
    for fn in nc.m.functions:
        for blk in fn.blocks:
            new = []
            changed = False
            for inst in blk.instructions:
                si = inst.sync_info
                waits = list(si.on_wait) if si is not None else []
                if len(waits) > max_waits:
                    changed = True
                    for w in waits[:-max_waits]:
                        ctr[0] += 1
                        new.append(mybir.InstNoOp(
                            name=f"I-waitsplit-{ctr[0]}",
                            engine=inst.engine,
                            sync_info=mybir.SyncInfo(on_wait=[w], on_update=[]),
                        ))
                    inst.sync_info = mybir.SyncInfo(
                        on_wait=waits[-max_waits:],
                        on_update=list(si.on_update),
                    )
                new.append(inst)
            if changed:
                blk.instructions = new


def _build_nc(has_beta):
    import concourse.bass as bass
    import concourse.tile as tile
    from concourse import mybir
    from concourse.masks import make_identity

    _patch_tile_drain()

    f32 = mybir.dt.float32
    f32r = mybir.dt.float32r
    bf16 = mybir.dt.bfloat16

    nc = bass.Bass()

    # All big inputs are host-pre-tiled to [128, ...] so every DMA line is
    # contiguous per partition.
    xT = nc.declare_dram_parameter("xT", [128, DT * TOK], bf16, isOutput=False)
    ctxT = nc.declare_dram_parameter("ctxT", [128, DKT * TOK], bf16, isOutput=False)
    wq = nc.declare_dram_parameter("wq", [128, DT * INNER], bf16, isOutput=False)
    wk = nc.declare_dram_parameter("wk", [128, DKT * INNER], bf16, isOutput=False)
    wv = nc.declare_dram_parameter("wv", [128, DKT * INNER], bf16, isOutput=False)
    wo = nc.declare_dram_parameter("wo", [128, DT * Q_DIM], bf16, isOutput=False)
    wsum_neg = nc.declare_dram_parameter("wsum_neg", [1, INNER], f32r, isOutput=False)
    bias_q = nc.declare_dram_parameter("bias_q", [1, INNER], f32r, isOutput=False)
    bo = nc.declare_dram_parameter("bo", [1, Q_DIM], f32, isOutput=False)
    ones_in = nc.declare_dram_parameter("ones_in", [1, 128], f32r, isOutput=False)
    y = nc.declare_dram_parameter("y", [TOK, Q_DIM], f32, isOutput=True)

    with tile.TileContext(nc) as tc:
        import contextlib

        with contextlib.ExitStack() as ctx:
            singles = ctx.enter_context(tc.tile_pool(name="singles", bufs=1))
            pt_pool = ctx.enter_context(tc.tile_pool(name="pt", bufs=5))
            osb_pool = ctx.enter_context(tc.tile_pool(name="osb", bufs=3))
            rc_pool = ctx.enter_context(tc.tile_pool(name="rc", bufs=4))
            tmp_pool = ctx.enter_context(tc.tile_pool(name="tmp", bufs=2))
            xsq_pool = ctx.enter_context(tc.tile_pool(name="xsq", bufs=2))
            y_pool = ctx.enter_context(tc.tile_pool(name="y", bufs=2))
            ps_proj = ctx.enter_context(
                tc.tile_pool(name="ps_proj", bufs=2, space="PSUM")
            )
            ps_st = ctx.enter_context(tc.tile_pool(name="ps_st", bufs=4, space="PSUM"))
            ps_av = ctx.enter_context(tc.tile_pool(name="ps_av", bufs=2, space="PSUM"))

            # ---- resident inputs -------------------------------------------
            xT_sb = singles.tile([128, DT, TOK], bf16)
            nc.sync.dma_start(out=xT_sb, in_=xT.rearrange("p (a t) -> p a t", t=TOK))
            ctxT_sb = singles.tile([128, DKT, TOK], bf16)
            nc.sync.dma_start(
                out=ctxT_sb, in_=ctxT.rearrange("p (a t) -> p a t", t=TOK)
            )
            wq_sb = singles.tile([128, DT, INNER], bf16)
            nc.sync.dma_start(out=wq_sb, in_=wq.rearrange("p (a j) -> p a j", j=INNER))
            wk_sb = singles.tile([128, DKT, INNER], bf16)
            nc.scalar.dma_start(
                out=wk_sb, in_=wk.rearrange("p (a j) -> p a j", j=INNER)
            )
            wv_sb = singles.tile([128, DKT, INNER], bf16)
            nc.scalar.dma_start(
                out=wv_sb, in_=wv.rearrange("p (a j) -> p a j", j=INNER)
            )
            wo_sb = singles.tile([128, DT, Q_DIM], bf16)
            nc.scalar.dma_start(
                out=wo_sb, in_=wo.rearrange("p (a j) -> p a j", j=Q_DIM)
            )
            wsum_sb = singles.tile([1, INNER], f32r)
            nc.sync.dma_start(out=wsum_sb, in_=wsum_neg[:, :])
            if has_beta:
                biasq_sb = singles.tile([1, INNER], f32r)
                nc.sync.dma_start(out=biasq_sb, in_=bias_q[:, :])
            bo_sb = singles.tile([128, Q_DIM], f32)
            nc.sync.dma_start(out=bo_sb, in_=bo[:, :].to_broadcast([128, Q_DIM]))
            ones_col = singles.tile([1, 128], f32r)
            nc.sync.dma_start(out=ones_col, in_=ones_in[:, :])

            # ---- constants -------------------------------------------------
            ones_inv_d = singles.tile([128, 1], bf16)
            nc.vector.memset(ones_inv_d, 1.0 / Q_DIM)
            eps_sb = singles.tile([1, 1], f32)
            nc.vector.memset(eps_sb, LN_EPS)
            ident = singles.tile([128, 128], bf16)
            make_identity(nc, ident)

            # ---- whole-core tensors ---------------------------------------
            QT_all = singles.tile([128, DT, TOK], bf16)
            KT_all = singles.tile([128, DT, TOK], bf16)
            V_all = singles.tile([128, NB * 2, HEADS * 65], bf16)
            OT_sb = singles.tile([128, DT, TOK], bf16)
            mu_sb = singles.tile([1, TOK], f32r)
            var_sb = singles.tile([1, TOK], f32r)
            rstd_sb = singles.tile([1, TOK], f32r)
            if has_beta:
                rinv_sb = singles.tile([1, TOK], f32r)
            rbc_sb = singles.tile([128, TOK], f32)

            Exp = mybir.ActivationFunctionType.Exp
            Sqrt = mybir.ActivationFunctionType.Sqrt

            # ---- phase A: LN statistics over all 1024 tokens ---------------
            for half in range(2):
                sl = slice(half * 512, (half + 1) * 512)
                mups = ps_proj.tile([1, 512], f32, tag="proj")
                for kt in range(DT):
                    nc.tensor.matmul(
                        mups, ones_inv_d, xT_sb[:, kt, sl],
                        start=(kt == 0), stop=(kt == DT - 1),
                    )
                nc.vector.tensor_copy(mu_sb[:, sl], mups)
                sqps = ps_proj.tile([1, 512], f32, tag="proj")
                for kt in range(DT):
                    xsq = xsq_pool.tile([128, 512], bf16)
                    nc.vector.tensor_mul(
                        xsq, xT_sb[:, kt, sl], xT_sb[:, kt, sl]
                    )
                    nc.tensor.matmul(
                        sqps, ones_inv_d, xsq,
                        start=(kt == 0), stop=(kt == DT - 1),
                    )
                nc.vector.tensor_copy(var_sb[:, sl], sqps)  # mean(x^2)
            for half in range(2):
                sl = slice(half * 512, (half + 1) * 512)
                musq = tmp_pool.tile([1, 512], f32, tag="musq")
                nc.vector.tensor_mul(musq, mu_sb[:, sl], mu_sb[:, sl])
                nc.vector.tensor_sub(var_sb[:, sl], var_sb[:, sl], musq)
                sqv = tmp_pool.tile([1, 512], f32, tag="sqv")
                nc.scalar.activation(sqv, var_sb[:, sl], Sqrt, bias=eps_sb)
                if has_beta:
                    nc.vector.tensor_copy(rinv_sb[:, sl], sqv)
                with nc.allow_low_precision(reason="fp32r rounding for PE"):
                    nc.vector.reciprocal(out=rstd_sb[:, sl], in_=sqv)
                rbcps = ps_proj.tile([128, 512], f32, tag="proj")
                nc.tensor.matmul(
                    rbcps, ones_col, rstd_sb[:, sl], start=True, stop=True
                )
                nc.vector.tensor_copy(rbc_sb[:, sl], rbcps)

            # ---- phase B: Q^T projection (LN folded in) --------------------
            for jt in range(DT):
                js = jt * 128
                for half in range(2):
                    sl = slice(half * 512, (half + 1) * 512)
                    qps = ps_proj.tile([128, 512], f32, tag="proj")
                    for kt in range(DT):
                        nc.tensor.matmul(
                            qps, wq_sb[:, kt, js:js + 128], xT_sb[:, kt, sl],
                            start=(kt == 0), stop=False,
                        )
                    nc.tensor.matmul(
                        qps, wsum_sb[:, js:js + 128], mu_sb[:, sl],
                        start=False, stop=(not has_beta),
                    )
                    if has_beta:
                        nc.tensor.matmul(
                            qps, biasq_sb[:, js:js + 128], rinv_sb[:, sl],
                            start=False, stop=True,
                        )
                    nc.vector.tensor_mul(
                        QT_all[:, jt, sl], qps, rbc_sb[:, sl]
                    )

            # ---- phase C: K^T projection -----------------------------------
            for jt in range(DT):
                js = jt * 128
                for half in range(2):
                    sl = slice(half * 512, (half + 1) * 512)
                    kps = ps_proj.tile([128, 512], f32, tag="proj")
                    for kt in range(DKT):
                        nc.tensor.matmul(
                            kps, wk_sb[:, kt, js:js + 128], ctxT_sb[:, kt, sl],
                            start=(kt == 0), stop=(kt == DKT - 1),
                        )
                    nc.scalar.copy(KT_all[:, jt, sl], kps)

            # ---- phase D: V projection (token-major, ones col at 64) -------
            nc.gpsimd.memset(
                V_all.rearrange("p t (h c) -> p t h c", c=65)[:, :, :, 64:65], 1.0
            )
            for tokt in range(NB * 2):
                cs = tokt * 128
                for jn in range(2):
                    vps = ps_proj.tile([128, 512], f32, tag="proj")
                    for kt in range(DKT):
                        nc.tensor.matmul(
                            vps,
                            ctxT_sb[:, kt, cs:cs + 128],
                            wv_sb[:, kt, jn * 512:(jn + 1) * 512],
                            start=(kt == 0), stop=(kt == DKT - 1),
                        )
                    nc.vector.tensor_copy(
                        V_all.rearrange("p t (h c) -> p t h c", c=65)[
                            :, tokt, jn * 8:(jn + 1) * 8, 0:64
                        ],
                        vps.rearrange("p (h c) -> p h c", c=64),
                    )

            # ---- phase E: per-block attention + out-projection -------------
            for b in range(NB):
                ts = b * BLK
                osb = [
                    osb_pool.tile([128, INNER], bf16, name=f"osb{t}", tag="osb")
                    for t in range(2)
                ]
                for hg in range(4):
                    sts = [
                        ps_st.tile([128, 512], f32, tag="stps", name=f"st{i}")
                        for i in range(4)
                    ]
                    # S^T: interleave heads so PE row-groups alternate 0/64
                    for t2t in range(2):
                        for hh in range(4):
                            h = hg * 4 + hh
                            jt, po = h // 2, (h % 2) * 64
                            nc.tensor.matmul(
                                sts[hh][:, t2t * BLK:(t2t + 1) * BLK],
                                KT_all[po:po + 64, jt,
                                       ts + t2t * 128:ts + (t2t + 1) * 128],
                                QT_all[po:po + 64, jt, ts:ts + BLK],
                                start=True, stop=True,
                            )
                    pts = []
                    for hh in range(4):
                        pt = pt_pool.tile([128, 512], bf16, tag="pt", name="pt")
                        nc.scalar.activation(pt, sts[hh], Exp, scale=SCALE)
                        pts.append(pt)
                    for t1t in range(2):
                        avp = ps_av.tile([128, 260], f32)
                        for hh in range(4):
                            h = hg * 4 + hh
                            for t2t in range(2):
                                nc.tensor.matmul(
                                    avp[:, hh * 65:(hh + 1) * 65],
                                    pts[hh][:, t2t * BLK + t1t * 128:
                                            t2t * BLK + (t1t + 1) * 128],
                                    V_all[:, 2 * b + t2t, h * 65:(h + 1) * 65],
                                    start=(t2t == 0), stop=(t2t == 1),
                                )
                        avp_h = avp.rearrange("p (h c) -> p h c", c=65)
                        rc = rc_pool.tile([128, 4], f32)
                        nc.vector.reciprocal(
                            out=rc,
                            in_=avp_h[:, :, 64:65].rearrange("p h c -> p (h c)"),
                        )
                        rc_b = rc.rearrange("p (h o) -> p h o", o=1).to_broadcast(
                            [128, 4, 64]
                        )
                        nc.vector.tensor_mul(
                            osb[t1t].rearrange("p (h c) -> p h c", c=64)[
                                :, hg * 4:(hg + 1) * 4, :
                            ],
                            avp_h[:, :, 0:64],
                            rc_b,
                        )

                # transpose O -> O^T
                for t1t in range(2):
                    for jt in range(DT):
                        trp = ps_st.tile([128, 128], bf16, tag="stps")
                        nc.tensor.transpose(
                            trp, osb[t1t][:, jt * 128:(jt + 1) * 128], ident
                        )
                        dst = OT_sb[:, jt, ts + t1t * 128: ts + (t1t + 1) * 128]
                        if jt % 2 == 0:
                            nc.vector.tensor_copy(dst, trp)
                        else:
                            nc.scalar.copy(dst, trp)

                # out-projection for this block's two token tiles
                for mtl in range(2):
                    mt = 2 * b + mtl
                    ms = mt * 128
                    y_sb = y_pool.tile([128, Q_DIM], f32)
                    for on in range(2):
                        yps = ps_proj.tile([128, 512], f32, tag="proj")
                        for kt in range(DT):
                            nc.tensor.matmul(
                                yps,
                                OT_sb[:, kt, ms:ms + 128],
                                wo_sb[:, kt, on * 512:(on + 1) * 512],
                                start=(kt == 0), stop=(kt == DT - 1),
                            )
                        nc.vector.tensor_add(
                            y_sb[:, on * 512:(on + 1) * 512],
                            yps,
                            bo_sb[:, on * 512:(on + 1) * 512],
                        )
                    nc.gpsimd.dma_start(out=y[ms:ms + 128, :], in_=y_sb)

    _split_multi_waits(nc, mybir)
    return nc


def _expected_mask():
    fid = np.repeat(np.arange(NUM_FRAMES), NUM_PATCHES)
    return (fid[:, None] == fid[None, :])[None, None]


def _reference_fallback(x, context, ln_gamma, ln_beta, Wq, Wkv, Wo, bo, mask):
    """Pure-numpy fallback for a non-block-diagonal mask (correctness only)."""
    x64 = x.astype(np.float64)
    mu = x64.mean(-1, keepdims=True)
    var = ((x64 - mu) ** 2).mean(-1, keepdims=True)
    xn = (x64 - mu) / np.sqrt(var + LN_EPS) * ln_gamma + ln_beta
    q = xn @ Wq.astype(np.float64)
    kv = context.astype(np.float64) @ Wkv.astype(np.float64)
    k, v = kv[..., :INNER], kv[..., INNER:]
    sh = lambda t: t.reshape(B, T, HEADS, DIM_HEAD).transpose(0, 2, 1, 3)
    q, k, v = sh(q), sh(k), sh(v)
    dots = np.einsum("bhnd,bhmd->bhnm", q, k) * SCALE
    dots = np.where(mask, dots, -np.inf)
    dots -= dots.max(-1, keepdims=True)
    e = np.exp(dots)
    attn = e / e.sum(-1, keepdims=True)
    out = np.einsum("bhnm,bhmd->bhnd", attn, v)
    out = out.transpose(0, 2, 1, 3).reshape(B, T, INNER)
    return (out @ Wo.astype(np.float64) + bo).astype(np.float32)


def _tile128(a):
    """[R, C] -> [128, (R/128)*C] partition-major pre-tiling for one-shot
    contiguous DMA into an SBUF [128, R/128, C] tile."""
    r, c = a.shape
    return np.ascontiguousarray(
        a.reshape(r // 128, 128, c).transpose(1, 0, 2).reshape(128, -1)
    )


def _prep_in_maps(x, context, ln_gamma, ln_beta, Wq, Wkv, Wo, bo):
    import ml_dtypes

    bf = ml_dtypes.bfloat16
    wq_eff = (ln_gamma[:, None] * Wq).astype(np.float32)
    wsum_neg = (-wq_eff.sum(axis=0, dtype=np.float64)).astype(np.float32)[None, :]
    bias_q = (ln_beta @ Wq).astype(np.float32)[None, :]
    wq_t = _tile128(wq_eff.astype(bf))
    wk_t = _tile128(np.ascontiguousarray(Wkv[:, :INNER]).astype(bf))
    wv_t = _tile128(np.ascontiguousarray(Wkv[:, INNER:]).astype(bf))
    wo_t = _tile128(Wo.astype(bf))
    bo2 = bo.astype(np.float32)[None, :]
    ones128 = np.ones((1, 128), np.float32)

    x_flat = x.reshape(B * T, Q_DIM)
    c_flat = context.reshape(B * T, KV_DIM)
    in_maps = []
    for c in range(N_CORES):
        sl = slice(c * TOK, (c + 1) * TOK)
        xT_t = _tile128(np.ascontiguousarray(x_flat[sl].T.astype(bf)))
        ctxT_t = _tile128(np.ascontiguousarray(c_flat[sl].T.astype(bf)))
        in_maps.append({
            "xT": xT_t, "ctxT": ctxT_t,
            "wq": wq_t, "wk": wk_t, "wv": wv_t, "wo": wo_t,
            "wsum_neg": wsum_neg, "bias_q": bias_q, "bo": bo2,
            "ones_in": ones128,
        })
    return in_maps


def _run(inputs, trace=False):
    from concourse.bass_utils import run_bass_kernel_spmd

    has_beta = bool(np.any(np.asarray(inputs["ln_beta"])))
    key = ("nc", has_beta)
    if key not in _CACHE:
        _CACHE[key] = _build_nc(has_beta)
    nc = _CACHE[key]
    in_maps = _prep_in_maps(
        inputs["x"], inputs["context"], inputs["ln_gamma"], inputs["ln_beta"],
        inputs["Wq"], inputs["Wkv"], inputs["Wo"], inputs["bo"],
    )
    res = run_bass_kernel_spmd(nc, in_maps, list(range(N_CORES)), trace=trace)
    y = np.concatenate([res.results[c]["y"] for c in range(N_CORES)], axis=0)
    return y.reshape(B, T, Q_DIM).astype(np.float32), res


def kernel(x, context, ln_gamma, ln_beta, Wq, Wkv, Wo, bo, mask):
    mask = np.asarray(mask)
    if not np.array_equal(mask, _expected_mask()):
        return _reference_fallback(
            np.asarray(x), np.asarray(context), np.asarray(ln_gamma),
            np.asarray(ln_beta), np.asarray(Wq), np.asarray(Wkv),
            np.asarray(Wo), np.asarray(bo), mask,
        )
    inputs = dict(x=np.asarray(x), context=np.asarray(context),
                  ln_gamma=np.asarray(ln_gamma), ln_beta=np.asarray(ln_beta),
                  Wq=np.asarray(Wq), Wkv=np.asarray(Wkv), Wo=np.asarray(Wo),
                  bo=np.asarray(bo))
    out, _ = _run(inputs, trace=False)
    return out


def _install_profiling_shims():
    """Enable the NTFF profile path under axon in this trimmed container:
    provide the antenv.axon_hooks registry and stub the artifact upload."""
    import sys
    import types

    if "antenv.axon_hooks" not in sys.modules:
        import antenv

        mod = types.ModuleType("antenv.axon_hooks")
        mod._hook = None

        def set_axon_ntff_profile_hook(h):
            mod._hook = h

        def get_axon_ntff_profile_hook():
            return mod._hook

        mod.set_axon_ntff_profile_hook = set_axon_ntff_profile_hook
        mod.get_axon_ntff_profile_hook = get_axon_ntff_profile_hook
        sys.modules["antenv.axon_hooks"] = mod
        antenv.axon_hooks = mod

    mod = sys.modules["antenv.axon_hooks"]
    if mod._hook is None:
        from trn_agent_boot.trn_boot import _ntff_profile_via_ctypes

        mod.set_axon_ntff_profile_hook(
            _ntff_profile_via_ctypes("/opt/axon/libaxon_pjrt.so")
        )

    from concourse import bass_utils

    if not getattr(bass_utils, "_upload_stubbed", False):
        bass_utils.upload_artifacts = lambda tmpdir: tmpdir
        bass_utils._upload_stubbed = True


def kernel_traced(**inputs):
    """Like kernel() but returns (out, BassKernelResults) with profiling."""
    _install_profiling_shims()
    out, res = _run(inputs, trace=True)
    return out, res


# revision 21
# speedup vs baseline: 1.2128x; 1.0505x over previous
"""Trainium2 Bass kernel for CrossAttentionInjection (block-diagonal frame attention).

Contract: kernel(**inputs) takes FULL unsharded numpy inputs (as produced by
setup_inputs()) and returns the FULL [B, T, Q_DIM] float32 output.

Sharding: the attention mask is block-diagonal over 8 frames x 256 patches, so
the whole module decomposes into 32 independent (batch, frame) blocks of 256
tokens. Each of the 8 cores processes 4 contiguous blocks (1024 tokens of one
batch) with replicated weights -- no collectives.

Per-core pipeline (bf16 matmuls, fp32 PSUM, fp32r rank-1 LN corrections):
  LN stats via ones-matmul rows; LN folded into the Q projection (gamma folded
  into Wq on the host, mean/beta as K=1 rank-1 matmuls, 1/std applied in the
  PSUM->SBUF multiply); per-head block attention with exp on ACT (softmax
  scale folded into the activation), denominator via a ones-column appended to
  V; per-partition normalize; PE-transpose of O for the output projection;
  bo added during the final PSUM->SBUF copy.
"""

import numpy as np

# ---------------------------------------------------------------------------
# Problem constants (hardcoded; kernel.py must be self-contained)
# ---------------------------------------------------------------------------
B, T, Q_DIM, KV_DIM = 4, 2048, 1024, 768
HEADS, DIM_HEAD = 16, 64
INNER = HEADS * DIM_HEAD  # 1024
NUM_FRAMES, NUM_PATCHES = 8, 256
LN_EPS = 1e-5
N_CORES = 8
TOK = B * T // N_CORES          # 1024 tokens per core
NB = TOK // NUM_PATCHES         # 4 frame-blocks per core
BLK = NUM_PATCHES               # 256
DT = Q_DIM // 128               # 8 q-dim partition tiles
DKT = KV_DIM // 128             # 6 kv-dim partition tiles
SCALE = DIM_HEAD ** -0.5        # 0.125

_CACHE = {}


def _patch_tile_drain():
    """This walrus build rejects >1 sync-wait on a Drain CTRL instruction.
    Split the Tile end-of-context drain waits across single-wait NOPs."""
    import concourse.tile as tile
    from concourse import mybir
    from concourse.vector_clock import ScopedClock

    if getattr(tile.TileContext, "_drain_patched", False):
        return

    def _drain_and_barrier(self, tick_clock, wait_clock):
        nc = self.nc
        probe = nc.sync.nop(nofuse=True)
        wait_clock.add_sem_waits(
            probe.ins, ScopedClock({None: tick_clock.global_clock})
        )
        si = probe.ins.sync_info
        waits = list(si.on_wait) if si is not None else []
        if waits:
            probe.ins.sync_info = mybir.SyncInfo(on_wait=[waits[0]], on_update=[])
            for w in waits[1:]:
                n = nc.sync.nop(nofuse=True)
                n.ins.sync_info = mybir.SyncInfo(on_wait=[w], on_update=[])
        nc.sync.drain()
        nc.all_engine_barrier()
        assert self.sems is not None
        popped = nc._tile_sem_poison_stack.pop()
        assert popped is self._sem_poison
        nc.clear_and_free_semaphores(list(self.sems.allocated().values()))
        nc.all_engine_barrier()

    tile.TileContext._drain_and_barrier = _drain_and_barrier
    tile.TileContext._drain_patched = True


def _split_multi_waits(nc, mybir, max_waits=1):
    """This walrus build accepts at most one sync-wait per instruction.
    Move extra waits onto single-wait NOPs inserted just before, on the
    same engine (sound: same-engine program order is preserved)."""
    ctr = [0]
    for fn in nc.m.functions:
        for blk in fn.blocks:
            new = []
            changed = False
            for inst in blk.instructions:
                si = inst.sync_info
                waits = list(si.on_wait) if si is not None else []
                if len(waits) > max_waits:
                    changed = True
                    for w in waits[:-max_waits]:
                        ctr[0] += 1
                        new.append(mybir.InstNoOp(
                            name=f"I-waitsplit-{ctr[0]}",
                            engine=inst.engine,
                            sync_info=mybir.SyncInfo(on_wait=[w], on_update=[]),
                        ))
                    inst.sync_info = mybir.SyncInfo(
                        on_wait=waits[-max_waits:],
                        on_update=list(si.on_update),
                    )
                new.append(inst)
            if changed:
                blk.instructions = new


def _build_nc(has_beta):
    import concourse.bass as bass
    import concourse.tile as tile
    from concourse import mybir
    from concourse.masks import make_identity

    _patch_tile_drain()

    f32 = mybir.dt.float32
    f32r = mybir.dt.float32r
    bf16 = mybir.dt.bfloat16

    nc = bass.Bass()

    # All big inputs are host-pre-tiled to [128, ...] so every DMA line is
    # contiguous per partition.
    xT = nc.declare_dram_parameter("xT", [128, DT * TOK], bf16, isOutput=False)
    ctxT = nc.declare_dram_parameter("ctxT", [128, DKT * TOK], bf16, isOutput=False)
    wq = nc.declare_dram_parameter("wq", [128, DT * INNER], bf16, isOutput=False)
    wk = nc.declare_dram_parameter("wk", [128, DKT * INNER], bf16, isOutput=False)
    wv = nc.declare_dram_parameter("wv", [128, DKT * INNER], bf16, isOutput=False)
    wo = nc.declare_dram_parameter("wo", [128, DT * Q_DIM], bf16, isOutput=False)
    wsum_neg = nc.declare_dram_parameter("wsum_neg", [1, INNER], f32r, isOutput=False)
    bias_q = nc.declare_dram_parameter("bias_q", [1, INNER], f32r, isOutput=False)
    bo = nc.declare_dram_parameter("bo", [1, Q_DIM], f32, isOutput=False)
    ones_in = nc.declare_dram_parameter("ones_in", [1, 128], f32r, isOutput=False)
    y = nc.declare_dram_parameter("y", [TOK, Q_DIM], f32, isOutput=True)

    with tile.TileContext(nc) as tc:
        import contextlib

        with contextlib.ExitStack() as ctx:
            singles = ctx.enter_context(tc.tile_pool(name="singles", bufs=1))
            pt_pool = ctx.enter_context(tc.tile_pool(name="pt", bufs=5))
            osb_pool = ctx.enter_context(tc.tile_pool(name="osb", bufs=4))
            rc_pool = ctx.enter_context(tc.tile_pool(name="rc", bufs=6))
            tmp_pool = ctx.enter_context(tc.tile_pool(name="tmp", bufs=2))
            xsq_pool = ctx.enter_context(tc.tile_pool(name="xsq", bufs=2))
            y_pool = ctx.enter_context(tc.tile_pool(name="y", bufs=2))
            ps_proj = ctx.enter_context(
                tc.tile_pool(name="ps_proj", bufs=2, space="PSUM")
            )
            ps_st = ctx.enter_context(tc.tile_pool(name="ps_st", bufs=4, space="PSUM"))
            ps_av = ctx.enter_context(tc.tile_pool(name="ps_av", bufs=2, space="PSUM"))

            # ---- resident inputs -------------------------------------------
            xT_sb = singles.tile([128, DT, TOK], bf16)
            nc.sync.dma_start(out=xT_sb, in_=xT.rearrange("p (a t) -> p a t", t=TOK))
            ctxT_sb = singles.tile([128, DKT, TOK], bf16)
            nc.sync.dma_start(
                out=ctxT_sb, in_=ctxT.rearrange("p (a t) -> p a t", t=TOK)
            )
            wq_sb = singles.tile([128, DT, INNER], bf16)
            nc.sync.dma_start(out=wq_sb, in_=wq.rearrange("p (a j) -> p a j", j=INNER))
            wk_sb = singles.tile([128, DKT, INNER], bf16)
            nc.scalar.dma_start(
                out=wk_sb, in_=wk.rearrange("p (a j) -> p a j", j=INNER)
            )
            wv_sb = singles.tile([128, DKT, INNER], bf16)
            nc.scalar.dma_start(
                out=wv_sb, in_=wv.rearrange("p (a j) -> p a j", j=INNER)
            )
            wo_sb = singles.tile([128, DT, Q_DIM], bf16)
            nc.scalar.dma_start(
                out=wo_sb, in_=wo.rearrange("p (a j) -> p a j", j=Q_DIM)
            )
            wsum_sb = singles.tile([1, INNER], f32r)
            nc.sync.dma_start(out=wsum_sb, in_=wsum_neg[:, :])
            if has_beta:
                biasq_sb = singles.tile([1, INNER], f32r)
                nc.sync.dma_start(out=biasq_sb, in_=bias_q[:, :])
            bo_sb = singles.tile([128, Q_DIM], f32)
            nc.sync.dma_start(out=bo_sb, in_=bo[:, :].to_broadcast([128, Q_DIM]))
            ones_col = singles.tile([1, 128], f32r)
            nc.sync.dma_start(out=ones_col, in_=ones_in[:, :])

            # ---- constants -------------------------------------------------
            ones_inv_d = singles.tile([128, 1], bf16)
            nc.vector.memset(ones_inv_d, 1.0 / Q_DIM)
            eps_sb = singles.tile([1, 1], f32)
            nc.vector.memset(eps_sb, LN_EPS)
            ident = singles.tile([128, 128], bf16)
            make_identity(nc, ident)

            # ---- whole-core tensors ---------------------------------------
            QT_all = singles.tile([128, DT, TOK], bf16)
            KT_all = singles.tile([128, DT, TOK], bf16)
            V_all = singles.tile([128, NB * 2, HEADS * 65], bf16)
            OT_sb = singles.tile([128, DT, TOK], bf16)
            mu_sb = singles.tile([1, TOK], f32r)
            var_sb = singles.tile([1, TOK], f32r)
            rstd_sb = singles.tile([1, TOK], f32r)
            if has_beta:
                rinv_sb = singles.tile([1, TOK], f32r)
            rbc_sb = singles.tile([128, TOK], f32)

            Exp = mybir.ActivationFunctionType.Exp
            Sqrt = mybir.ActivationFunctionType.Sqrt

            # ---- phase A: LN statistics over all 1024 tokens ---------------
            for half in range(2):
                sl = slice(half * 512, (half + 1) * 512)
                mups = ps_proj.tile([1, 512], f32, tag="proj")
                for kt in range(DT):
                    nc.tensor.matmul(
                        mups, ones_inv_d, xT_sb[:, kt, sl],
                        start=(kt == 0), stop=(kt == DT - 1),
                    )
                nc.vector.tensor_copy(mu_sb[:, sl], mups)
                sqps = ps_proj.tile([1, 512], f32, tag="proj")
                for kt in range(DT):
                    xsq = xsq_pool.tile([128, 512], bf16)
                    nc.vector.tensor_mul(
                        xsq, xT_sb[:, kt, sl], xT_sb[:, kt, sl]
                    )
                    nc.tensor.matmul(
                        sqps, ones_inv_d, xsq,
                        start=(kt == 0), stop=(kt == DT - 1),
                    )
                nc.vector.tensor_copy(var_sb[:, sl], sqps)  # mean(x^2)
            for half in range(2):
                sl = slice(half * 512, (half + 1) * 512)
                musq = tmp_pool.tile([1, 512], f32, tag="musq")
                nc.vector.tensor_mul(musq, mu_sb[:, sl], mu_sb[:, sl])
                nc.vector.tensor_sub(var_sb[:, sl], var_sb[:, sl], musq)
                sqv = tmp_pool.tile([1, 512], f32, tag="sqv")
                nc.scalar.activation(sqv, var_sb[:, sl], Sqrt, bias=eps_sb)
                if has_beta:
                    nc.vector.tensor_copy(rinv_sb[:, sl], sqv)
                with nc.allow_low_precision(reason="fp32r rounding for PE"):
                    nc.vector.reciprocal(out=rstd_sb[:, sl], in_=sqv)
                rbcps = ps_proj.tile([128, 512], f32, tag="proj")
                nc.tensor.matmul(
                    rbcps, ones_col, rstd_sb[:, sl], start=True, stop=True
                )
                nc.vector.tensor_copy(rbc_sb[:, sl], rbcps)

            # ---- phase B: Q^T projection (LN folded in) --------------------
            for jt in range(DT):
                js = jt * 128
                for half in range(2):
                    sl = slice(half * 512, (half + 1) * 512)
                    qps = ps_proj.tile([128, 512], f32, tag="proj")
                    for kt in range(DT):
                        nc.tensor.matmul(
                            qps, wq_sb[:, kt, js:js + 128], xT_sb[:, kt, sl],
                            start=(kt == 0), stop=False,
                        )
                    nc.tensor.matmul(
                        qps, wsum_sb[:, js:js + 128], mu_sb[:, sl],
                        start=False, stop=(not has_beta),
                    )
                    if has_beta:
                        nc.tensor.matmul(
                            qps, biasq_sb[:, js:js + 128], rinv_sb[:, sl],
                            start=False, stop=True,
                        )
                    nc.vector.tensor_mul(
                        QT_all[:, jt, sl], qps, rbc_sb[:, sl]
                    )

            # ---- phase C: K^T projection -----------------------------------
            for jt in range(DT):
                js = jt * 128
                for half in range(2):
                    sl = slice(half * 512, (half + 1) * 512)
                    kps = ps_proj.tile([128, 512], f32, tag="proj")
                    for kt in range(DKT):
                        nc.tensor.matmul(
                            kps, wk_sb[:, kt, js:js + 128], ctxT_sb[:, kt, sl],
                            start=(kt == 0), stop=(kt == DKT - 1),
                        )
                    nc.scalar.copy(KT_all[:, jt, sl], kps)

            # ---- phase D: V projection (token-major, ones col at 64) -------
            nc.gpsimd.memset(
                V_all.rearrange("p t (h c) -> p t h c", c=65)[:, :, :, 64:65], 1.0
            )
            for tokt in range(NB * 2):
                cs = tokt * 128
                for jn in range(2):
                    vps = ps_proj.tile([128, 512], f32, tag="proj")
                    for kt in range(DKT):
                        nc.tensor.matmul(
                            vps,
                            ctxT_sb[:, kt, cs:cs + 128],
                            wv_sb[:, kt, jn * 512:(jn + 1) * 512],
                            start=(kt == 0), stop=(kt == DKT - 1),
                        )
                    nc.vector.tensor_copy(
                        V_all.rearrange("p t (h c) -> p t h c", c=65)[
                            :, tokt, jn * 8:(jn + 1) * 8, 0:64
                        ],
                        vps.rearrange("p (h c) -> p h c", c=64),
                    )

            # ---- phase E: per-block attention + out-projection -------------
            for b in range(NB):
                ts = b * BLK
                osb = [
                    osb_pool.tile([128, INNER], bf16, name=f"osb{t}", tag="osb")
                    for t in range(2)
                ]
                for hg in range(8):
                    sts = [
                        ps_st.tile([128, 512], f32, tag="stps", name=f"st{i}")
                        for i in range(2)
                    ]
                    # S^T: interleave heads so PE row-groups alternate 0/64
                    for t2t in range(2):
                        for hh in range(2):
                            h = hg * 2 + hh
                            jt, po = h // 2, (h % 2) * 64
                            nc.tensor.matmul(
                                sts[hh][:, t2t * BLK:(t2t + 1) * BLK],
                                KT_all[po:po + 64, jt,
                                       ts + t2t * 128:ts + (t2t + 1) * 128],
                                QT_all[po:po + 64, jt, ts:ts + BLK],
                                start=True, stop=True,
                            )
                    pts = []
                    for hh in range(2):
                        pt = pt_pool.tile([128, 512], bf16, tag="pt", name="pt")
                        nc.scalar.activation(pt, sts[hh], Exp, scale=SCALE)
                        pts.append(pt)
                    for t1t in range(2):
                        avp = ps_av.tile([128, 130], f32)
                        for hh in range(2):
                            h = hg * 2 + hh
                            for t2t in range(2):
                                nc.tensor.matmul(
                                    avp[:, hh * 65:(hh + 1) * 65],
                                    pts[hh][:, t2t * BLK + t1t * 128:
                                            t2t * BLK + (t1t + 1) * 128],
                                    V_all[:, 2 * b + t2t, h * 65:(h + 1) * 65],
                                    start=(t2t == 0), stop=(t2t == 1),
                                )
                        avp_h = avp.rearrange("p (h c) -> p h c", c=65)
                        rc = rc_pool.tile([128, 2], f32)
                        nc.vector.reciprocal(
                            out=rc,
                            in_=avp_h[:, :, 64:65].rearrange("p h c -> p (h c)"),
                        )
                        rc_b = rc.rearrange("p (h o) -> p h o", o=1).to_broadcast(
                            [128, 2, 64]
                        )
                        nc.vector.tensor_mul(
                            osb[t1t].rearrange("p (h c) -> p h c", c=64)[
                                :, hg * 2:(hg + 1) * 2, :
                            ],
                            avp_h[:, :, 0:64],
                            rc_b,
                        )

                # transpose O -> O^T
                for t1t in range(2):
                    for jt in range(DT):
                        trp = ps_st.tile([128, 128], bf16, tag="stps")
                        nc.tensor.transpose(
                            trp, osb[t1t][:, jt * 128:(jt + 1) * 128], ident
                        )
                        dst = OT_sb[:, jt, ts + t1t * 128: ts + (t1t + 1) * 128]
                        if jt % 2 == 0:
                            nc.vector.tensor_copy(dst, trp)
                        else:
                            nc.scalar.copy(dst, trp)

                # out-projection for this block's two token tiles
                for mtl in range(2):
                    mt = 2 * b + mtl
                    ms = mt * 128
                    y_sb = y_pool.tile([128, Q_DIM], f32)
                    for on in range(2):
                        yps = ps_proj.tile([128, 512], f32, tag="proj")
                        for kt in range(DT):
                            nc.tensor.matmul(
                                yps,
                                OT_sb[:, kt, ms:ms + 128],
                                wo_sb[:, kt, on * 512:(on + 1) * 512],
                                start=(kt == 0), stop=(kt == DT - 1),
                            )
                        nc.vector.tensor_add(
                            y_sb[:, on * 512:(on + 1) * 512],
                            yps,
                            bo_sb[:, on * 512:(on + 1) * 512],
                        )
                    nc.gpsimd.dma_start(out=y[ms:ms + 128, :], in_=y_sb)

    _split_multi_waits(nc, mybir)
    return nc


def _expected_mask():
    fid = np.repeat(np.arange(NUM_FRAMES), NUM_PATCHES)
    return (fid[:, None] == fid[None, :])[None, None]


def _reference_fallback(x, context, ln_gamma, ln_beta, Wq, Wkv, Wo, bo, mask):
    """Pure-numpy fallback for a non-block-diagonal mask (correctness only)."""
    x64 = x.astype(np.float64)
    mu = x64.mean(-1, keepdims=True)
    var = ((x64 - mu) ** 2).mean(-1, keepdims=True)
    xn = (x64 - mu) / np.sqrt(var + LN_EPS) * ln_gamma + ln_beta
    q = xn @ Wq.astype(np.float64)
    kv = context.astype(np.float64) @ Wkv.astype(np.float64)
    k, v = kv[..., :INNER], kv[..., INNER:]
    sh = lambda t: t.reshape(B, T, HEADS, DIM_HEAD).transpose(0, 2, 1, 3)
    q, k, v = sh(q), sh(k), sh(v)
    dots = np.einsum("bhnd,bhmd->bhnm", q, k) * SCALE
    dots = np.where(mask, dots, -np.inf)
    dots -= dots.max(-1, keepdims=True)
    e = np.exp(dots)
    attn = e / e.sum(-1, keepdims=True)
    out = np.einsum("bhnm,bhmd->bhnd", attn, v)
    out = out.transpose(0, 2, 1, 3).reshape(B, T, INNER)
    return (out @ Wo.astype(np.float64) + bo).astype(np.float32)


def _tile128(a):
    """[R, C] -> [128, (R/128)*C] partition-major pre-tiling for one-shot
    contiguous DMA into an SBUF [128, R/128, C] tile."""
    r, c = a.shape
    return np.ascontiguousarray(
        a.reshape(r // 128, 128, c).transpose(1, 0, 2).reshape(128, -1)
    )


def _prep_in_maps(x, context, ln_gamma, ln_beta, Wq, Wkv, Wo, bo):
    import ml_dtypes

    bf = ml_dtypes.bfloat16
    wq_eff = (ln_gamma[:, None] * Wq).astype(np.float32)
    wsum_neg = (-wq_eff.sum(axis=0, dtype=np.float64)).astype(np.float32)[None, :]
    bias_q = (ln_beta @ Wq).astype(np.float32)[None, :]
    wq_t = _tile128(wq_eff.astype(bf))
    wk_t = _tile128(np.ascontiguousarray(Wkv[:, :INNER]).astype(bf))
    wv_t = _tile128(np.ascontiguousarray(Wkv[:, INNER:]).astype(bf))
    wo_t = _tile128(Wo.astype(bf))
    bo2 = bo.astype(np.float32)[None, :]
    ones128 = np.ones((1, 128), np.float32)

    x_flat = x.reshape(B * T, Q_DIM)
    c_flat = context.reshape(B * T, KV_DIM)
    in_maps = []
    for c in range(N_CORES):
        sl = slice(c * TOK, (c + 1) * TOK)
        xT_t = _tile128(np.ascontiguousarray(x_flat[sl].T.astype(bf)))
        ctxT_t = _tile128(np.ascontiguousarray(c_flat[sl].T.astype(bf)))
        in_maps.append({
            "xT": xT_t, "ctxT": ctxT_t,
            "wq": wq_t, "wk": wk_t, "wv": wv_t, "wo": wo_t,
            "wsum_neg": wsum_neg, "bias_q": bias_q, "bo": bo2,
            "ones_in": ones128,
        })
    return in_maps


def _run(inputs, trace=False):
    from concourse.bass_utils import run_bass_kernel_spmd

    has_beta = bool(np.any(np.asarray(inputs["ln_beta"])))
    key = ("nc", has_beta)
    if key not in _CACHE:
        _CACHE[key] = _build_nc(has_beta)
    nc = _CACHE[key]
    in_maps = _prep_in_maps(
        inputs["x"], inputs["context"], inputs["ln_gamma"], inputs["ln_beta"],
        inputs["Wq"], inputs["Wkv"], inputs["Wo"], inputs["bo"],
    )
    res = run_bass_kernel_spmd(nc, in_maps, list(range(N_CORES)), trace=trace)
    y = np.concatenate([res.results[c]["y"] for c in range(N_CORES)], axis=0)
    return y.reshape(B, T, Q_DIM).astype(np.float32), res


def kernel(x, context, ln_gamma, ln_beta, Wq, Wkv, Wo, bo, mask):
    mask = np.asarray(mask)
    if not np.array_equal(mask, _expected_mask()):
        return _reference_fallback(
            np.asarray(x), np.asarray(context), np.asarray(ln_gamma),
            np.asarray(ln_beta), np.asarray(Wq), np.asarray(Wkv),
            np.asarray(Wo), np.asarray(bo), mask,
        )
    inputs = dict(x=np.asarray(x), context=np.asarray(context),
                  ln_gamma=np.asarray(ln_gamma), ln_beta=np.asarray(ln_beta),
                  Wq=np.asarray(Wq), Wkv=np.asarray(Wkv), Wo=np.asarray(Wo),
                  bo=np.asarray(bo))
    out, _ = _run(inputs, trace=False)
    return out


def _install_profiling_shims():
    """Enable the NTFF profile path under axon in this trimmed container:
    provide the antenv.axon_hooks registry and stub the artifact upload."""
    import sys
    import types

    if "antenv.axon_hooks" not in sys.modules:
        import antenv

        mod = types.ModuleType("antenv.axon_hooks")
        mod._hook = None

        def set_axon_ntff_profile_hook(h):
            mod._hook = h

        def get_axon_ntff_profile_hook():
            return mod._hook

        mod.set_axon_ntff_profile_hook = set_axon_ntff_profile_hook
        mod.get_axon_ntff_profile_hook = get_axon_ntff_profile_hook
        sys.modules["antenv.axon_hooks"] = mod
        antenv.axon_hooks = mod

    mod = sys.modules["antenv.axon_hooks"]
    if mod._hook is None:
        from trn_agent_boot.trn_boot import _ntff_profile_via_ctypes

        mod.set_axon_ntff_profile_hook(
            _ntff_profile_via_ctypes("/opt/axon/libaxon_pjrt.so")
        )

    from concourse import bass_utils

    if not getattr(bass_utils, "_upload_stubbed", False):
        bass_utils.upload_artifacts = lambda tmpdir: tmpdir
        bass_utils._upload_stubbed = True


def kernel_traced(**inputs):
    """Like kernel() but returns (out, BassKernelResults) with profiling."""
    _install_profiling_shims()
    out, res = _run(inputs, trace=True)
    return out, res


# revision 24
# speedup vs baseline: 1.2302x; 1.0143x over previous
"""Trainium2 Bass kernel for CrossAttentionInjection (block-diagonal frame attention).

Contract: kernel(**inputs) takes FULL unsharded numpy inputs (as produced by
setup_inputs()) and returns the FULL [B, T, Q_DIM] float32 output.

Sharding: the attention mask is block-diagonal over 8 frames x 256 patches, so
the whole module decomposes into 32 independent (batch, frame) blocks of 256
tokens. Each of the 8 cores processes 4 contiguous blocks (1024 tokens of one
batch) with replicated weights -- no collectives.

Per-core pipeline (bf16 matmuls, fp32 PSUM, fp32r rank-1 LN corrections):
  LN stats via ones-matmul rows; LN folded into the Q projection (gamma folded
  into Wq on the host, mean/beta as K=1 rank-1 matmuls, 1/std applied in the
  PSUM->SBUF multiply); per-head block attention with exp on ACT (softmax
  scale folded into the activation), denominator via a ones-column appended to
  V; per-partition normalize; PE-transpose of O for the output projection;
  bo added during the final PSUM->SBUF copy.
"""

import numpy as np

# ---------------------------------------------------------------------------
# Problem constants (hardcoded; kernel.py must be self-contained)
# ---------------------------------------------------------------------------
B, T, Q_DIM, KV_DIM = 4, 2048, 1024, 768
HEADS, DIM_HEAD = 16, 64
INNER = HEADS * DIM_HEAD  # 1024
NUM_FRAMES, NUM_PATCHES = 8, 256
LN_EPS = 1e-5
N_CORES = 8
TOK = B * T // N_CORES          # 1024 tokens per core
NB = TOK // NUM_PATCHES         # 4 frame-blocks per core
BLK = NUM_PATCHES               # 256
DT = Q_DIM // 128               # 8 q-dim partition tiles
DKT = KV_DIM // 128             # 6 kv-dim partition tiles
SCALE = DIM_HEAD ** -0.5        # 0.125

_CACHE = {}


def _patch_tile_drain():
    """This walrus build rejects >1 sync-wait on a Drain CTRL instruction.
    Split the Tile end-of-context drain waits across single-wait NOPs."""
    import concourse.tile as tile
    from concourse import mybir
    from concourse.vector_clock import ScopedClock

    if getattr(tile.TileContext, "_drain_patched", False):
        return

    def _drain_and_barrier(self, tick_clock, wait_clock):
        nc = self.nc
        probe = nc.sync.nop(nofuse=True)
        wait_clock.add_sem_waits(
            probe.ins, ScopedClock({None: tick_clock.global_clock})
        )
        si = probe.ins.sync_info
        waits = list(si.on_wait) if si is not None else []
        if waits:
            probe.ins.sync_info = mybir.SyncInfo(on_wait=[waits[0]], on_update=[])
            for w in waits[1:]:
                n = nc.sync.nop(nofuse=True)
                n.ins.sync_info = mybir.SyncInfo(on_wait=[w], on_update=[])
        nc.sync.drain()
        nc.all_engine_barrier()
        assert self.sems is not None
        popped = nc._tile_sem_poison_stack.pop()
        assert popped is self._sem_poison
        nc.clear_and_free_semaphores(list(self.sems.allocated().values()))
        nc.all_engine_barrier()

    tile.TileContext._drain_and_barrier = _drain_and_barrier
    tile.TileContext._drain_patched = True


def _split_multi_waits(nc, mybir, max_waits=1):
    """This walrus build accepts at most one sync-wait per instruction.
    Move extra waits onto single-wait NOPs inserted just before, on the
    same engine (sound: same-engine program order is preserved)."""
    ctr = [0]
    for fn in nc.m.functions:
        for blk in fn.blocks:
            new = []
            changed = False
            for inst in blk.instructions:
                si = inst.sync_info
                waits = list(si.on_wait) if si is not None else []
                if len(waits) > max_waits:
                    changed = True
                    for w in waits[:-max_waits]:
                        ctr[0] += 1
                        new.append(mybir.InstNoOp(
                            name=f"I-waitsplit-{ctr[0]}",
                            engine=inst.engine,
                            sync_info=mybir.SyncInfo(on_wait=[w], on_update=[]),
                        ))
                    inst.sync_info = mybir.SyncInfo(
                        on_wait=waits[-max_waits:],
                        on_update=list(si.on_update),
                    )
                new.append(inst)
            if changed:
                blk.instructions = new


def _build_nc(has_beta):
    import concourse.bass as bass
    import concourse.tile as tile
    from concourse import mybir
    from concourse.masks import make_identity

    _patch_tile_drain()

    f32 = mybir.dt.float32
    f32r = mybir.dt.float32r
    bf16 = mybir.dt.bfloat16

    nc = bass.Bass()

    # All big inputs are host-pre-tiled to [128, ...] so every DMA line is
    # contiguous per partition.
    xT = nc.declare_dram_parameter("xT", [128, DT * TOK], bf16, isOutput=False)
    ctxT = nc.declare_dram_parameter("ctxT", [128, DKT * TOK], bf16, isOutput=False)
    wq = nc.declare_dram_parameter("wq", [128, DT * INNER], bf16, isOutput=False)
    wk = nc.declare_dram_parameter("wk", [128, DKT * INNER], bf16, isOutput=False)
    wv = nc.declare_dram_parameter("wv", [128, DKT * INNER], bf16, isOutput=False)
    wo = nc.declare_dram_parameter("wo", [128, DT * Q_DIM], bf16, isOutput=False)
    wsum_neg = nc.declare_dram_parameter("wsum_neg", [1, INNER], f32r, isOutput=False)
    bias_q = nc.declare_dram_parameter("bias_q", [1, INNER], f32r, isOutput=False)
    bo = nc.declare_dram_parameter("bo", [1, Q_DIM], f32, isOutput=False)
    ones_in = nc.declare_dram_parameter("ones_in", [1, 128], f32r, isOutput=False)
    y = nc.declare_dram_parameter("y", [TOK, Q_DIM], f32, isOutput=True)

    with tile.TileContext(nc) as tc:
        import contextlib

        with contextlib.ExitStack() as ctx:
            singles = ctx.enter_context(tc.tile_pool(name="singles", bufs=1))
            pt_pool = ctx.enter_context(tc.tile_pool(name="pt", bufs=4))
            osb_pool = ctx.enter_context(tc.tile_pool(name="osb", bufs=3))
            rc_pool = ctx.enter_context(tc.tile_pool(name="rc", bufs=6))
            tmp_pool = ctx.enter_context(tc.tile_pool(name="tmp", bufs=2))
            xsq_pool = ctx.enter_context(tc.tile_pool(name="xsq", bufs=2))
            y_pool = ctx.enter_context(tc.tile_pool(name="y", bufs=2))
            ps_proj = ctx.enter_context(
                tc.tile_pool(name="ps_proj", bufs=2, space="PSUM")
            )
            ps_st = ctx.enter_context(tc.tile_pool(name="ps_st", bufs=4, space="PSUM"))
            ps_av = ctx.enter_context(tc.tile_pool(name="ps_av", bufs=2, space="PSUM"))

            # ---- resident inputs -------------------------------------------
            xT_sb = singles.tile([128, DT, TOK], bf16)
            nc.sync.dma_start(out=xT_sb, in_=xT.rearrange("p (a t) -> p a t", t=TOK))
            ctxT_sb = singles.tile([128, DKT, TOK], bf16)
            nc.sync.dma_start(
                out=ctxT_sb, in_=ctxT.rearrange("p (a t) -> p a t", t=TOK)
            )
            wq_sb = singles.tile([128, DT, INNER], bf16)
            nc.sync.dma_start(out=wq_sb, in_=wq.rearrange("p (a j) -> p a j", j=INNER))
            wk_sb = singles.tile([128, DKT, INNER], bf16)
            nc.scalar.dma_start(
                out=wk_sb, in_=wk.rearrange("p (a j) -> p a j", j=INNER)
            )
            wv_sb = singles.tile([128, DKT, INNER], bf16)
            nc.scalar.dma_start(
                out=wv_sb, in_=wv.rearrange("p (a j) -> p a j", j=INNER)
            )
            wo_sb = singles.tile([128, DT, Q_DIM], bf16)
            nc.scalar.dma_start(
                out=wo_sb, in_=wo.rearrange("p (a j) -> p a j", j=Q_DIM)
            )
            if has_beta:
                biasq_sb = singles.tile([1, INNER], f32r)
                nc.sync.dma_start(out=biasq_sb, in_=bias_q[:, :])
            bo_sb = singles.tile([128, Q_DIM], f32)
            nc.sync.dma_start(out=bo_sb, in_=bo[:, :].to_broadcast([128, Q_DIM]))
            ones_col = singles.tile([1, 128], f32r)
            nc.sync.dma_start(out=ones_col, in_=ones_in[:, :])

            # ---- constants -------------------------------------------------
            ones_inv_d = singles.tile([128, 1], bf16)
            nc.vector.memset(ones_inv_d, 1.0 / Q_DIM)
            eps_sb = singles.tile([1, 1], f32)
            nc.vector.memset(eps_sb, LN_EPS)
            ident = singles.tile([128, 128], bf16)
            make_identity(nc, ident)

            # ---- whole-core tensors ---------------------------------------
            QT_all = singles.tile([128, DT, TOK], bf16)
            mu_bc_sb = singles.tile([128, TOK], f32)
            KT_all = singles.tile([128, DT, TOK], bf16)
            V_all = singles.tile([128, NB * 2, HEADS * 65], bf16)
            OT_sb = singles.tile([128, DT, TOK], bf16)
            mu_sb = singles.tile([1, TOK], f32r)
            var_sb = singles.tile([1, TOK], f32r)
            rstd_sb = singles.tile([1, TOK], f32r)
            if has_beta:
                rinv_sb = singles.tile([1, TOK], f32r)
            rbc_sb = singles.tile([128, TOK], f32)

            Exp = mybir.ActivationFunctionType.Exp
            Sqrt = mybir.ActivationFunctionType.Sqrt

            # ---- phase A: LN statistics over all 1024 tokens ---------------
            for half in range(2):
                sl = slice(half * 512, (half + 1) * 512)
                mups = ps_proj.tile([1, 512], f32, tag="proj")
                for kt in range(DT):
                    nc.tensor.matmul(
                        mups, ones_inv_d, xT_sb[:, kt, sl],
                        start=(kt == 0), stop=(kt == DT - 1),
                    )
                nc.vector.tensor_copy(mu_sb[:, sl], mups)
                sqps = ps_proj.tile([1, 512], f32, tag="proj")
                for kt in range(DT):
                    xsq = xsq_pool.tile([128, 512], bf16)
                    nc.vector.tensor_mul(
                        xsq, xT_sb[:, kt, sl], xT_sb[:, kt, sl]
                    )
                    nc.tensor.matmul(
                        sqps, ones_inv_d, xsq,
                        start=(kt == 0), stop=(kt == DT - 1),
                    )
                nc.vector.tensor_copy(var_sb[:, sl], sqps)  # mean(x^2)
            for half in range(2):
                sl = slice(half * 512, (half + 1) * 512)
                musq = tmp_pool.tile([1, 512], f32, tag="musq")
                nc.vector.tensor_mul(musq, mu_sb[:, sl], mu_sb[:, sl])
                nc.vector.tensor_sub(var_sb[:, sl], var_sb[:, sl], musq)
                sqv = tmp_pool.tile([1, 512], f32, tag="sqv")
                nc.scalar.activation(sqv, var_sb[:, sl], Sqrt, bias=eps_sb)
                if has_beta:
                    nc.vector.tensor_copy(rinv_sb[:, sl], sqv)
                with nc.allow_low_precision(reason="fp32r rounding for PE"):
                    nc.vector.reciprocal(out=rstd_sb[:, sl], in_=sqv)
                rbcps = ps_proj.tile([128, 512], f32, tag="proj")
                nc.tensor.matmul(
                    rbcps, ones_col, rstd_sb[:, sl], start=True, stop=True
                )
                nc.vector.tensor_copy(rbc_sb[:, sl], rbcps)
                mbps = ps_proj.tile([128, 512], f32, tag="proj")
                nc.tensor.matmul(
                    mbps, ones_col, mu_sb[:, sl], start=True, stop=True
                )
                nc.vector.tensor_copy(mu_bc_sb[:, sl], mbps)

            # ---- phase C: K^T projection -----------------------------------
            for jt in range(DT):
                js = jt * 128
                for half in range(2):
                    sl = slice(half * 512, (half + 1) * 512)
                    kps = ps_proj.tile([128, 512], f32, tag="proj")
                    for kt in range(DKT):
                        nc.tensor.matmul(
                            kps, wk_sb[:, kt, js:js + 128], ctxT_sb[:, kt, sl],
                            start=(kt == 0), stop=(kt == DKT - 1),
                        )
                    nc.scalar.copy(KT_all[:, jt, sl], kps)

            # ---- phase D: V projection (token-major, ones col at 64) -------
            nc.gpsimd.memset(
                V_all.rearrange("p t (h c) -> p t h c", c=65)[:, :, :, 64:65], 1.0
            )
            for tokt in range(NB * 2):
                cs = tokt * 128
                for jn in range(2):
                    vps = ps_proj.tile([128, 512], f32, tag="proj")
                    for kt in range(DKT):
                        nc.tensor.matmul(
                            vps,
                            ctxT_sb[:, kt, cs:cs + 128],
                            wv_sb[:, kt, jn * 512:(jn + 1) * 512],
                            start=(kt == 0), stop=(kt == DKT - 1),
                        )
                    nc.vector.tensor_copy(
                        V_all.rearrange("p t (h c) -> p t h c", c=65)[
                            :, tokt, jn * 8:(jn + 1) * 8, 0:64
                        ],
                        vps.rearrange("p (h c) -> p h c", c=64),
                    )

            # ---- phase B (after V): x - mu, then Q^T projection ------------
            for kt in range(DT):
                for half in range(2):
                    sl = slice(half * 512, (half + 1) * 512)
                    nc.vector.tensor_sub(
                        xT_sb[:, kt, sl], xT_sb[:, kt, sl], mu_bc_sb[:, sl]
                    )
            for jt in range(DT):
                js = jt * 128
                for half in range(2):
                    sl = slice(half * 512, (half + 1) * 512)
                    qps = ps_proj.tile([128, 512], f32, tag="proj")
                    for kt in range(DT):
                        nc.tensor.matmul(
                            qps, wq_sb[:, kt, js:js + 128], xT_sb[:, kt, sl],
                            start=(kt == 0), stop=(kt == DT - 1 and not has_beta),
                        )
                    if has_beta:
                        nc.tensor.matmul(
                            qps, biasq_sb[:, js:js + 128], rinv_sb[:, sl],
                            start=False, stop=True,
                        )
                    nc.vector.tensor_mul(
                        QT_all[:, jt, sl], qps, rbc_sb[:, sl]
                    )

            # ---- phase E: per-block attention + out-projection -------------
            for b in range(NB):
                ts = b * BLK
                osb = [
                    osb_pool.tile([128, INNER], bf16, name=f"osb{t}", tag="osb")
                    for t in range(2)
                ]
                for hg in range(8):
                    sts = [
                        ps_st.tile([128, 512], f32, tag="stps", name=f"st{i}")
                        for i in range(2)
                    ]
                    # S^T: interleave heads so PE row-groups alternate 0/64
                    for t2t in range(2):
                        for hh in range(2):
                            h = hg * 2 + hh
                            jt, po = h // 2, (h % 2) * 64
                            nc.tensor.matmul(
                                sts[hh][:, t2t * BLK:(t2t + 1) * BLK],
                                KT_all[po:po + 64, jt,
                                       ts + t2t * 128:ts + (t2t + 1) * 128],
                                QT_all[po:po + 64, jt, ts:ts + BLK],
                                start=True, stop=True,
                            )
                    pts = []
                    for hh in range(2):
                        pt = pt_pool.tile([128, 512], bf16, tag="pt", name="pt")
                        nc.scalar.activation(pt, sts[hh], Exp, scale=SCALE)
                        pts.append(pt)
                    for t1t in range(2):
                        avp = ps_av.tile([128, 130], f32)
                        for hh in range(2):
                            h = hg * 2 + hh
                            for t2t in range(2):
                                nc.tensor.matmul(
                                    avp[:, hh * 65:(hh + 1) * 65],
                                    pts[hh][:, t2t * BLK + t1t * 128:
                                            t2t * BLK + (t1t + 1) * 128],
                                    V_all[:, 2 * b + t2t, h * 65:(h + 1) * 65],
                                    start=(t2t == 0), stop=(t2t == 1),
                                )
                        avp_h = avp.rearrange("p (h c) -> p h c", c=65)
                        rc = rc_pool.tile([128, 2], f32)
                        nc.vector.reciprocal(
                            out=rc,
                            in_=avp_h[:, :, 64:65].rearrange("p h c -> p (h c)"),
                        )
                        rc_b = rc.rearrange("p (h o) -> p h o", o=1).to_broadcast(
                            [128, 2, 64]
                        )
                        nc.vector.tensor_mul(
                            osb[t1t].rearrange("p (h c) -> p h c", c=64)[
                                :, hg * 2:(hg + 1) * 2, :
                            ],
                            avp_h[:, :, 0:64],
                            rc_b,
                        )

                # transpose O -> O^T
                for t1t in range(2):
                    for jt in range(DT):
                        trp = ps_st.tile([128, 128], bf16, tag="stps")
                        nc.tensor.transpose(
                            trp, osb[t1t][:, jt * 128:(jt + 1) * 128], ident
                        )
                        dst = OT_sb[:, jt, ts + t1t * 128: ts + (t1t + 1) * 128]
                        if jt % 2 == 0:
                            nc.vector.tensor_copy(dst, trp)
                        else:
                            nc.scalar.copy(dst, trp)

                # out-projection for this block's two token tiles
                for mtl in range(2):
                    mt = 2 * b + mtl
                    ms = mt * 128
                    y_sb = y_pool.tile([128, Q_DIM], f32)
                    for on in range(2):
                        yps = ps_proj.tile([128, 512], f32, tag="proj")
                        for kt in range(DT):
                            nc.tensor.matmul(
                                yps,
                                OT_sb[:, kt, ms:ms + 128],
                                wo_sb[:, kt, on * 512:(on + 1) * 512],
                                start=(kt == 0), stop=(kt == DT - 1),
                            )
                        nc.vector.tensor_add(
                            y_sb[:, on * 512:(on + 1) * 512],
                            yps,
                            bo_sb[:, on * 512:(on + 1) * 512],
                        )
                    nc.gpsimd.dma_start(out=y[ms:ms + 128, :], in_=y_sb)

    _split_multi_waits(nc, mybir)
    return nc


def _expected_mask():
    fid = np.repeat(np.arange(NUM_FRAMES), NUM_PATCHES)
    return (fid[:, None] == fid[None, :])[None, None]


def _reference_fallback(x, context, ln_gamma, ln_beta, Wq, Wkv, Wo, bo, mask):
    """Pure-numpy fallback for a non-block-diagonal mask (correctness only)."""
    x64 = x.astype(np.float64)
    mu = x64.mean(-1, keepdims=True)
    var = ((x64 - mu) ** 2).mean(-1, keepdims=True)
    xn = (x64 - mu) / np.sqrt(var + LN_EPS) * ln_gamma + ln_beta
    q = xn @ Wq.astype(np.float64)
    kv = context.astype(np.float64) @ Wkv.astype(np.float64)
    k, v = kv[..., :INNER], kv[..., INNER:]
    sh = lambda t: t.reshape(B, T, HEADS, DIM_HEAD).transpose(0, 2, 1, 3)
    q, k, v = sh(q), sh(k), sh(v)
    dots = np.einsum("bhnd,bhmd->bhnm", q, k) * SCALE
    dots = np.where(mask, dots, -np.inf)
    dots -= dots.max(-1, keepdims=True)
    e = np.exp(dots)
    attn = e / e.sum(-1, keepdims=True)
    out = np.einsum("bhnm,bhmd->bhnd", attn, v)
    out = out.transpose(0, 2, 1, 3).reshape(B, T, INNER)
    return (out @ Wo.astype(np.float64) + bo).astype(np.float32)


def _tile128(a):
    """[R, C] -> [128, (R/128)*C] partition-major pre-tiling for one-shot
    contiguous DMA into an SBUF [128, R/128, C] tile."""
    r, c = a.shape
    return np.ascontiguousarray(
        a.reshape(r // 128, 128, c).transpose(1, 0, 2).reshape(128, -1)
    )


def _prep_in_maps(x, context, ln_gamma, ln_beta, Wq, Wkv, Wo, bo):
    import ml_dtypes

    bf = ml_dtypes.bfloat16
    wq_eff = (ln_gamma[:, None] * Wq).astype(np.float32)
    wsum_neg = (-wq_eff.sum(axis=0, dtype=np.float64)).astype(np.float32)[None, :]
    bias_q = (ln_beta @ Wq).astype(np.float32)[None, :]
    wq_t = _tile128(wq_eff.astype(bf))
    wk_t = _tile128(np.ascontiguousarray(Wkv[:, :INNER]).astype(bf))
    wv_t = _tile128(np.ascontiguousarray(Wkv[:, INNER:]).astype(bf))
    wo_t = _tile128(Wo.astype(bf))
    bo2 = bo.astype(np.float32)[None, :]
    ones128 = np.ones((1, 128), np.float32)

    x_flat = x.reshape(B * T, Q_DIM)
    c_flat = context.reshape(B * T, KV_DIM)
    in_maps = []
    for c in range(N_CORES):
        sl = slice(c * TOK, (c + 1) * TOK)
        xT_t = _tile128(np.ascontiguousarray(x_flat[sl].T.astype(bf)))
        ctxT_t = _tile128(np.ascontiguousarray(c_flat[sl].T.astype(bf)))
        in_maps.append({
            "xT": xT_t, "ctxT": ctxT_t,
            "wq": wq_t, "wk": wk_t, "wv": wv_t, "wo": wo_t,
            "wsum_neg": wsum_neg, "bias_q": bias_q, "bo": bo2,
            "ones_in": ones128,
        })
    return in_maps


def _run(inputs, trace=False):
    from concourse.bass_utils import run_bass_kernel_spmd

    has_beta = bool(np.any(np.asarray(inputs["ln_beta"])))
    key = ("nc", has_beta)
    if key not in _CACHE:
        _CACHE[key] = _build_nc(has_beta)
    nc = _CACHE[key]
    in_maps = _prep_in_maps(
        inputs["x"], inputs["context"], inputs["ln_gamma"], inputs["ln_beta"],
        inputs["Wq"], inputs["Wkv"], inputs["Wo"], inputs["bo"],
    )
    res = run_bass_kernel_spmd(nc, in_maps, list(range(N_CORES)), trace=trace)
    y = np.concatenate([res.results[c]["y"] for c in range(N_CORES)], axis=0)
    return y.reshape(B, T, Q_DIM).astype(np.float32), res


def kernel(x, context, ln_gamma, ln_beta, Wq, Wkv, Wo, bo, mask):
    mask = np.asarray(mask)
    if not np.array_equal(mask, _expected_mask()):
        return _reference_fallback(
            np.asarray(x), np.asarray(context), np.asarray(ln_gamma),
            np.asarray(ln_beta), np.asarray(Wq), np.asarray(Wkv),
            np.asarray(Wo), np.asarray(bo), mask,
        )
    inputs = dict(x=np.asarray(x), context=np.asarray(context),
                  ln_gamma=np.asarray(ln_gamma), ln_beta=np.asarray(ln_beta),
                  Wq=np.asarray(Wq), Wkv=np.asarray(Wkv), Wo=np.asarray(Wo),
                  bo=np.asarray(bo))
    out, _ = _run(inputs, trace=False)
    return out


def _install_profiling_shims():
    """Enable the NTFF profile path under axon in this trimmed container:
    provide the antenv.axon_hooks registry and stub the artifact upload."""
    import sys
    import types

    if "antenv.axon_hooks" not in sys.modules:
        import antenv

        mod = types.ModuleType("antenv.axon_hooks")
        mod._hook = None

        def set_axon_ntff_profile_hook(h):
            mod._hook = h

        def get_axon_ntff_profile_hook():
            return mod._hook

        mod.set_axon_ntff_profile_hook = set_axon_ntff_profile_hook
        mod.get_axon_ntff_profile_hook = get_axon_ntff_profile_hook
        sys.modules["antenv.axon_hooks"] = mod
        antenv.axon_hooks = mod

    mod = sys.modules["antenv.axon_hooks"]
    if mod._hook is None:
        from trn_agent_boot.trn_boot import _ntff_profile_via_ctypes

        mod.set_axon_ntff_profile_hook(
            _ntff_profile_via_ctypes("/opt/axon/libaxon_pjrt.so")
        )

    from concourse import bass_utils

    if not getattr(bass_utils, "_upload_stubbed", False):
        bass_utils.upload_artifacts = lambda tmpdir: tmpdir
        bass_utils._upload_stubbed = True


def kernel_traced(**inputs):
    """Like kernel() but returns (out, BassKernelResults) with profiling."""
    _install_profiling_shims()
    out, res = _run(inputs, trace=True)
    return out, res
